# revision 20
# baseline (speedup 1.0000x reference)
"""Trainium2 Bass kernel for a PointNet-style neighborhood encoder.

Computation (matches the reference nn.Module):
    h = relu(relu(relu(points @ W0 + b0) @ W1 + b1) @ W2 + b2)   # [N,3] -> [N,128]
    pooled = segment_max(h, cluster)                             # [C,128], 32 pts/cluster
    out = relu(relu(pooled @ G0 + g0) @ G1 + g1)                 # [C,256]

Sharding: data-parallel over points across 8 NeuronCores (cluster
boundaries are shard-aligned because clusters are contiguous, 32
points each). Weights are replicated. No collectives; the host
scatters inputs and gathers per-core outputs.

Device strategy (per core, n = 262144 points = 65536 quad-columns):
  - Host packs points feature-major, 4 points per 128-partition column
    ("quads"): pts4[3a+f, q] = points[4q+a, f], so layer 0 is a single
    block-diagonal matmul (K=12, M=128) producing h0 for 4 points/col.
  - Layer 1 uses two permuted block-diagonal stationaries W1A/W1B
    (K=128, M=128) producing h1 with 2 points per column.
  - Layer 2 uses W2 duplicated on both partition halves; 4 sub-matmuls
    (K=64, M=128) with rhs partition slices map to distinct PE row
    groups, producing z = W2^T h1 (bias/relu deferred) in PSUM.
  - segment_max: relu is monotone and b2 is constant per feature, so
    pooled = relu(max_p(z) + b2). max over (4 tensors x 8 quads) is ONE
    VectorE tensor_reduce(axis=XY) straight out of PSUM per sub-chunk.
  - ScalarE (ACT) does every relu+bias PSUM->SBUF evacuation; VectorE
    only does the pooling reduces. bf16 activations everywhere
    (PSUM stays f32 as the HW requires).
  - Global MLP on pooled [128, 8192] per core; output is written
    feature-major [256, 8192] bf16 and transposed/upcast on the host.
"""

import numpy as np

# ---- problem geometry (hardcoded per contract) ----
N = 2097152          # total points
C = 65536            # clusters
PTS = 32             # points per cluster
NCORES = 8
NPC = N // NCORES    # points per core = 262144
N4C = NPC // 4       # quad-columns per core = 65536
CPC = C // NCORES    # clusters per core = 8192

BIG = 1024           # quad-columns per big-chunk
SUB = 256            # quad-columns per L2/pool sub-chunk
NCHUNK = N4C // BIG  # 64
NSUB = BIG // SUB    # 4

_CACHE = {}


def _bf16():
    import ml_dtypes
    return ml_dtypes.bfloat16


def _build_module(n4c: int):
    """Build the Bass module (SPMD program, same for all cores)."""
    import concourse.bass as bass
    import concourse.bacc as bacc
    import concourse.tile as tile
    from concourse import mybir

    BF = mybir.dt.bfloat16
    F32 = mybir.dt.float32
    RELU = mybir.ActivationFunctionType.Relu
    IDENT = mybir.ActivationFunctionType.Identity
    MAX = mybir.AluOpType.max
    XY = mybir.AxisListType.XY

    nchunk = n4c // BIG
    cpc = n4c // 8          # clusters per core for this size

    nc = bacc.Bacc()

    # ---- DRAM I/O ----
    pts4 = nc.dram_tensor("pts4", [12, n4c], BF, kind="ExternalInput")
    # packed stationaries: w1a|w1b|w2d|g0w|g1lo|g1hi|w0q(rows 0:12)
    wpack = nc.dram_tensor("wpack", [128, 896], BF, kind="ExternalInput")
    # packed biases: b0q|b1d|b2v|g0v|g1l|g1h
    bpack = nc.dram_tensor("bpack", [128, 6], F32, kind="ExternalInput")
    outt = nc.dram_tensor("outt", [256, cpc], BF, kind="ExternalOutput")

    from contextlib import ExitStack
    with tile.TileContext(nc) as tc, ExitStack() as ctx:
        singles = ctx.enter_context(tc.tile_pool(name="singles", bufs=1))
        ppts = ctx.enter_context(tc.tile_pool(name="ppts", bufs=3))
        ph0s = ctx.enter_context(tc.tile_pool(name="ph0s", bufs=2))
        ph1s = ctx.enter_context(tc.tile_pool(name="ph1s", bufs=2))
        psum_h = ctx.enter_context(tc.tile_pool(name="psum_h", bufs=2, space="PSUM"))
        psum_h1 = ctx.enter_context(tc.tile_pool(name="psum_h1", bufs=1, space="PSUM"))
        psum_z = ctx.enter_context(tc.tile_pool(name="psum_z", bufs=2, space="PSUM"))

        # ---- load constants (two batched DMAs: SWDGE setup is ~1us/DMA) ----
        wpack_s = singles.tile([128, 896], BF)
        bpack_s = singles.tile([128, 6], F32)
        nc.sync.dma_start(out=wpack_s[:], in_=wpack[:])
        nc.sync.dma_start(out=bpack_s[:], in_=bpack[:])
        w1a_sv = wpack_s[:, 0:128]
        w1b_sv = wpack_s[:, 128:256]
        w2d_sv = wpack_s[:, 256:384]
        g0w_sv = wpack_s[:, 384:512]
        g1lo_sv = wpack_s[:, 512:640]
        g1hi_sv = wpack_s[:, 640:768]
        w0q_sv = wpack_s[0:12, 768:896]
        b0q_sv = bpack_s[:, 0:1]
        b1d_sv = bpack_s[:, 1:2]
        b2v_sv = bpack_s[:, 2:3]
        g0v_sv = bpack_s[:, 3:4]
        g1l_sv = bpack_s[:, 4:5]
        g1h_sv = bpack_s[:, 5:6]

        # PE warm-up: the cost model ramps the PE clock over ~3us from
        # the first matmul; a dummy matmul at t~0 starts that clock while
        # the input DMAs are still in flight.
        warm = singles.tile([128, 128], BF)
        nc.vector.memset(warm[:], 0.0)
        wrm = psum_h.tile([128, 128], F32, tag="h0p")
        nc.tensor.matmul(wrm[:], warm[0:16, :], warm[0:16, :])
        wsink = singles.tile([128, 1], F32)
        nc.vector.tensor_reduce(wsink[:], wrm[:], axis=mybir.AxisListType.X,
                                op=MAX)
        # preload the Relu activation table while the input DMAs run; the
        # implicit ACT_TABLE_LOAD (~1.3us) otherwise lands on the first
        # h0 evacuation, squarely on the pipeline-fill critical path.
        wact = singles.tile([128, 1], BF)
        nc.scalar.activation(wact[:], warm[:, 0:1], RELU)

        # pooled max(z) accumulator for the whole core
        pooled = singles.tile([128, cpc], BF)
        # t3-slot staging for the cross-chunk batched fold: per chunk, ACT
        # evacuates one zp tile's t3 slot here ([q][c] layout); every 4
        # chunks a packed-2x TT tree folds the batch into pooled.
        ebuf = singles.tile([128, 1024], BF)
        pfold = ctx.enter_context(tc.tile_pool(name="pfold", bufs=2))

        # chunks in all batches but the last get the t3-slot of their j=3
        # zp tile routed via ACT into ebuf (uses ACT slack; DVE reduce
        # shrinks); the tree fold amortizes across 4 chunks.
        # batched t3-fold disabled: the 4-deep serial fold chain on the
        # saturated DVE delays block-k g0in by ~5us, costing more than the
        # ~73ns/chunk egress saving it buys. Kept for reference; n_split=0
        # routes everything through the plain grouped reduces.
        n_split = 0

        # ---- main loop over point chunks (L2/pool shifted one chunk) ----
        def emit_l2_pair(i, jpair, h1as, h1bs):
            """L2 matmuls for subs (2*jpair, 2*jpair+1) of chunk i, weight-
            batched (both lo-half MMs, then both hi-half), then the pools."""
            zps = []
            for j in (2 * jpair, 2 * jpair + 1):
                s0 = j * SUB
                zp = psum_z.tile([128, 4 * SUB], F32, tag="zp")
                zps.append((j, s0, zp))
            for j, s0, zp in zps:
                nc.tensor.matmul(zp[:, 0:SUB],
                                 w2d_sv[0:64, :], h1as[0:64, s0:s0 + SUB])
                nc.tensor.matmul(zp[:, SUB:2 * SUB],
                                 w2d_sv[0:64, :], h1bs[0:64, s0:s0 + SUB])
            for j, s0, zp in zps:
                nc.tensor.matmul(zp[:, 2 * SUB:3 * SUB],
                                 w2d_sv[64:128, :], h1as[64:128, s0:s0 + SUB])
                nc.tensor.matmul(zp[:, 3 * SUB:4 * SUB],
                                 w2d_sv[64:128, :], h1bs[64:128, s0:s0 + SUB])
            for j, s0, zp in zps:
                base = i * (BIG // 8) + j * (SUB // 8)
                if j == 3 and i < n_split:
                    # t0..t2 on DVE; t3 staged to ebuf on ACT ([q][c] so
                    # the batch fold pairs packed halves)
                    zv = zp[:, 0:3 * SUB].rearrange(
                        "p (t c q) -> p c t q", t=3, q=8)
                    nc.vector.tensor_reduce(
                        pooled[:, base:base + SUB // 8], zv, axis=XY, op=MAX)
                    eb = ebuf[:, (i % 4) * 256:(i % 4) * 256 + 256]
                    ev = eb.rearrange("p (q c) -> p q c", q=8,
                                      c=32).transpose((0, 2, 1))
                    nc.scalar.activation(ev, zp[:, 3 * SUB:4 * SUB], IDENT)
                else:
                    # pooled_raw = max over (4 tensors x 8 quads) per cluster
                    zv = zp.rearrange("p (t c q) -> p c t q", t=4, q=8)
                    nc.vector.tensor_reduce(
                        pooled[:, base:base + SUB // 8], zv, axis=XY, op=MAX)

        def emit_batch_fold(i0):
            """Fold ebuf (4 chunks x t3-slot, [ch][q][c]) into pooled."""
            f1 = pfold.tile([128, 512], BF, tag="ef1")
            v0 = ebuf[:].rearrange("p (ch h r) -> p ch h r", ch=4, h=2, r=128)
            nc.vector.tensor_tensor(out=f1[:].rearrange(
                "p (ch r) -> p ch r", ch=4), in0=v0[:, :, 0, :],
                in1=v0[:, :, 1, :], op=MAX)
            f2 = pfold.tile([128, 256], BF, tag="ef2")
            v1 = f1[:].rearrange("p (ch h r) -> p ch h r", ch=4, h=2, r=64)
            nc.vector.tensor_tensor(out=f2[:].rearrange(
                "p (ch r) -> p ch r", ch=4), in0=v1[:, :, 0, :],
                in1=v1[:, :, 1, :], op=MAX)
            f3 = pfold.tile([128, 128], BF, tag="ef3")
            v2 = f2[:].rearrange("p (ch h r) -> p ch h r", ch=4, h=2, r=32)
            nc.vector.tensor_tensor(out=f3[:].rearrange(
                "p (ch r) -> p ch r", ch=4), in0=v2[:, :, 0, :],
                in1=v2[:, :, 1, :], op=MAX)
            # combine into the j=3 cluster slices of the 4 chunks
            pv = pooled[:, i0 * 128:(i0 + 4) * 128].rearrange(
                "p (ch c) -> p ch c", ch=4)[:, :, 96:128]
            f3v = f3[:].rearrange("p (ch c) -> p ch c", ch=4)
            nc.vector.tensor_tensor(out=pv, in0=pv, in1=f3v, op=MAX)

        g0in = singles.tile([128, cpc], BF)
        g1in = singles.tile([128, cpc], BF)
        goutL = singles.tile([128, cpc], BF)
        goutH = singles.tile([128, cpc], BF)

        def g_slice(k, h):
            return slice(k * 512 + h * 256, k * 512 + (h + 1) * 256)

        def g_task_g0(k, h, sl=None):
            sl = g_slice(k, h) if sl is None else sl
            n = sl.stop - sl.start
            nc.scalar.activation(g0in[:, sl], pooled[:, sl], RELU,
                                 bias=b2v_sv)
            gp = psum_h.tile([128, n], F32, tag="h0p")
            nc.tensor.matmul(gp[:], g0w_sv, g0in[:, sl])
            nc.scalar.activation(g1in[:, sl], gp[:], RELU, bias=g0v_sv)

        def g_task_lo(k, h, sl=None):
            sl = g_slice(k, h) if sl is None else sl
            n = sl.stop - sl.start
            gpl = psum_h.tile([128, n], F32, tag="h0p")
            nc.tensor.matmul(gpl[:], g1lo_sv, g1in[:, sl])
            nc.scalar.activation(goutL[:, sl], gpl[:], RELU, bias=g1l_sv)
            nc.sync.dma_start(out=outt[0:128, sl], in_=goutL[:, sl])

        def g_task_hi(k, h, sl=None):
            sl = g_slice(k, h) if sl is None else sl
            n = sl.stop - sl.start
            gph = psum_h.tile([128, n], F32, tag="h0p")
            nc.tensor.matmul(gph[:], g1hi_sv, g1in[:, sl])
            nc.scalar.activation(goutH[:, sl], gph[:], RELU, bias=g1h_sv)
            nc.gpsimd.dma_start(out=outt[128:256, sl], in_=goutH[:, sl])

        def emit_g_block(k):
            for h in (0, 1):
                g_task_g0(k, h); g_task_lo(k, h); g_task_hi(k, h)

        g_tasks = []

        def pop_g_task():
            if g_tasks:
                fn, k, h = g_tasks.pop(0)
                fn(k, h)

        prev = None   # (i, h1as, h1bs) pending L2+pool
        for i in range(nchunk):
            c0 = i * BIG
            pts_t = ppts.tile([12, BIG], BF)
            if i == 0:
                # Pool-engine DGE: runs while SP is still issuing wpack
                nc.gpsimd.dma_start(out=pts_t[:], in_=pts4[:, c0:c0 + BIG])
            else:
                nc.sync.dma_start(out=pts_t[:], in_=pts4[:, c0:c0 + BIG])

            # L0: 4-point block-diagonal matmul, K=12 -> M=128
            h0pa = psum_h.tile([128, 512], F32, tag="h0p")
            h0pb = psum_h.tile([128, 512], F32, tag="h0p")
            nc.tensor.matmul(h0pa[:], w0q_sv, pts_t[:, 0:512])
            nc.tensor.matmul(h0pb[:], w0q_sv, pts_t[:, 512:1024])
            h0s = ph0s.tile([128, BIG], BF)
            nc.scalar.activation(h0s[:, 0:512], h0pa[:], RELU, bias=b0q_sv)
            if i == 0:
                # pipeline fill: the second half on the (still idle) DVE
                nc.vector.tensor_scalar(out=h0s[:, 512:1024], in0=h0pb[:],
                                        scalar1=b0q_sv, scalar2=0.0,
                                        op0=mybir.AluOpType.add,
                                        op1=mybir.AluOpType.max)
            else:
                nc.scalar.activation(h0s[:, 512:1024], h0pb[:], RELU,
                                     bias=b0q_sv)

            if prev is not None:
                emit_l2_pair(prev[0], 0, prev[1], prev[2])
                pop_g_task()
                emit_l2_pair(prev[0], 1, prev[1], prev[2])
                if prev[0] % 4 == 3 and prev[0] < n_split:
                    emit_batch_fold(prev[0] - 3)

            # L1: two block-diagonal stationaries -> h1 (2 pts/col)
            h1p = psum_h1.tile([128, BIG], F32, tag="h1p")
            h1as = ph1s.tile([128, BIG], BF, tag="h1as")
            if i == 0:
                # pipeline fill: run both L1 stationaries on the first
                # 512 columns first (ACT and DVE evacuate in parallel) so
                # L2 pair 0 unblocks before the second halves even start.
                h1p2 = psum_z.tile([128, BIG], F32, tag="zp")
                h1bs = ph1s.tile([128, BIG], BF, tag="h1bs")
                nc.tensor.matmul(h1p[:, 0:512], w1a_sv, h0s[:, 0:512])
                nc.tensor.matmul(h1p2[:, 0:512], w1b_sv, h0s[:, 0:512])
                nc.scalar.activation(h1as[:, 0:512], h1p[:, 0:512], RELU,
                                     bias=b1d_sv)
                nc.vector.tensor_scalar(out=h1bs[:, 0:512],
                                        in0=h1p2[:, 0:512],
                                        scalar1=b1d_sv, scalar2=0.0,
                                        op0=mybir.AluOpType.add,
                                        op1=mybir.AluOpType.max)
                emit_l2_pair(0, 0, h1as, h1bs)
                nc.tensor.matmul(h1p[:, 512:1024], w1a_sv, h0s[:, 512:1024])
                nc.tensor.matmul(h1p2[:, 512:1024], w1b_sv,
                                 h0s[:, 512:1024])
                nc.scalar.activation(h1as[:, 512:1024], h1p[:, 512:1024],
                                     RELU, bias=b1d_sv)
                nc.vector.tensor_scalar(out=h1bs[:, 512:1024],
                                        in0=h1p2[:, 512:1024],
                                        scalar1=b1d_sv, scalar2=0.0,
                                        op0=mybir.AluOpType.add,
                                        op1=mybir.AluOpType.max)
            else:
                nc.tensor.matmul(h1p[:, 0:512], w1a_sv, h0s[:, 0:512])
                nc.tensor.matmul(h1p[:, 512:1024], w1a_sv,
                                 h0s[:, 512:1024])
                nc.scalar.activation(h1as[:], h1p[:], RELU, bias=b1d_sv)
                h1p2 = psum_h1.tile([128, BIG], F32, tag="h1p")
                nc.tensor.matmul(h1p2[:, 0:512], w1b_sv, h0s[:, 0:512])
                nc.tensor.matmul(h1p2[:, 512:1024], w1b_sv,
                                 h0s[:, 512:1024])
                h1bs = ph1s.tile([128, BIG], BF, tag="h1bs")
                nc.scalar.activation(h1bs[:], h1p2[:], RELU, bias=b1d_sv)

            prev = (i, h1as, h1bs)

            # interleave global-MLP work once pooled slices complete:
            # block k (clusters 512k..512k+512) is pooled after iteration
            # 4k+4 starts (the shifted L2 of chunk 4k+3 was emitted above).
            if i >= 4 and (i - 4) % 4 == 0:
                k = (i - 4) // 4
                g_tasks.extend([(f, k, h) for h in (0, 1)
                                for f in (g_task_g0, g_task_lo, g_task_hi)])
            # the final half-block (last_k, 0) only needs chunks up to
            # 4*last_k+1, all pooled once the last iteration's shifted L2
            # has been emitted.
            if i == nchunk - 1 and nchunk >= 8:
                lk = cpc // 512 - 1
                g_tasks.extend([(f, lk, 0)
                                for f in (g_task_g0, g_task_lo, g_task_hi)])
            pop_g_task()
            if i == 0:
                # pipeline fill: chunk 0's L2+pool goes out immediately so
                # the DVE reduce stream starts a chunk earlier (pair 0 was
                # emitted inside the interleaved L1 block above).
                emit_l2_pair(0, 1, h1as, h1bs)
                prev = None

        # epilogue: overlap the final global-MLP work with the last pools.
        # Half-block (k, 0) only needs chunks 4k..4k+1 (pooled in-loop), so
        # everything except the very last half-block can run alongside the
        # final L2 pairs; (last_k, 1) needs the last chunk's pools.
        emit_l2_pair(prev[0], 0, prev[1], prev[2])
        for fn, k, h in g_tasks:
            fn(k, h)
        first_unpushed = ((nchunk - 5) // 4 + 1) if nchunk >= 5 else 0
        last_k = cpc // 512 - 1
        for k in range(first_unpushed, last_k + 1):
            if not (k == last_k and nchunk >= 8):
                for f in (g_task_g0, g_task_lo, g_task_hi):
                    f(k, 0)
            if k < last_k:
                for f in (g_task_g0, g_task_lo, g_task_hi):
                    f(k, 1)
        slA = slice(last_k * 512 + 256, last_k * 512 + 448)
        slB = slice(last_k * 512 + 448, last_k * 512 + 512)
        for f in (g_task_g0, g_task_lo, g_task_hi):
            f(last_k, 1, sl=slA)
        emit_l2_pair(prev[0], 1, prev[1], prev[2])
        # last sub-block: hand-scheduled chain, cheapest engine per step
        ADD = mybir.AluOpType.add
        nB = slB.stop - slB.start
        nc.vector.tensor_scalar(out=g0in[:, slB], in0=pooled[:, slB],
                                scalar1=b2v_sv, scalar2=0.0,
                                op0=ADD, op1=MAX)
        gpB = psum_h.tile([128, nB], F32, tag="h0p")
        nc.tensor.matmul(gpB[:], g0w_sv, g0in[:, slB])
        nc.vector.tensor_scalar(out=g1in[:, slB], in0=gpB[:],
                                scalar1=g0v_sv, scalar2=0.0,
                                op0=ADD, op1=MAX)
        gpBl = psum_h.tile([128, nB], F32, tag="h0p")
        nc.tensor.matmul(gpBl[:], g1lo_sv, g1in[:, slB])
        gpBh = psum_h.tile([128, nB], F32, tag="h0p")
        nc.tensor.matmul(gpBh[:], g1hi_sv, g1in[:, slB])
        nc.scalar.activation(goutL[:, slB], gpBl[:], RELU, bias=g1l_sv)
        nc.sync.dma_start(out=outt[0:128, slB], in_=goutL[:, slB])
        nc.vector.tensor_scalar(out=goutH[:, slB], in0=gpBh[:],
                                scalar1=g1h_sv, scalar2=0.0,
                                op0=ADD, op1=MAX)
        nc.gpsimd.dma_start(out=outt[128:256, slB], in_=goutH[:, slB])

    nc.compile()
    return nc


def _host_pack(points, W0, b0, W1, b1, W2, b2, G0, g0, G1, g1, n4c):
    """Build per-core input maps (host-side layout prep, numpy only)."""
    bf16 = _bf16()
    n = n4c * 4 * NCORES

    # pts4[3a+f, q] = points[4q+a, f]
    pts4 = np.ascontiguousarray(
        points[:n].reshape(-1, 4, 3).transpose(1, 2, 0).reshape(12, -1)
    ).astype(bf16)

    # W0 block-diagonal over 4 points: [12, 128]
    w0q = np.zeros((12, 128), np.float32)
    for a in range(4):
        w0q[3 * a:3 * a + 3, 32 * a:32 * a + 32] = W0
    # W1A/W1B: rows 32a+f; cols 64a'+g ; a' in {0,1} / {2,3}
    w1a = np.zeros((128, 128), np.float32)
    w1b = np.zeros((128, 128), np.float32)
    for a in range(2):
        w1a[32 * a:32 * a + 32, 64 * a:64 * a + 64] = W1
        w1b[32 * (a + 2):32 * (a + 2) + 32, 64 * a:64 * a + 64] = W1
    # W2 duplicated on both partition halves
    w2d = np.concatenate([W2, W2], axis=0)

    wpack = np.zeros((128, 896), np.float32)
    wpack[:, 0:128] = w1a
    wpack[:, 128:256] = w1b
    wpack[:, 256:384] = w2d
    wpack[:, 384:512] = G0
    wpack[:, 512:640] = G1[:, :128]
    wpack[:, 640:768] = G1[:, 128:]
    wpack[0:12, 768:896] = w0q

    bpackm = np.zeros((128, 6), np.float32)
    bpackm[:, 0] = np.tile(b0, 4)
    bpackm[:, 1] = np.tile(b1, 2)
    bpackm[:, 2] = b2
    bpackm[:, 3] = g0
    bpackm[:, 4] = g1[:128]
    bpackm[:, 5] = g1[128:]

    common = {
        "wpack": wpack.astype(bf16),
        "bpack": bpackm,
    }
    in_maps = []
    for c in range(NCORES):
        m = dict(common)
        m["pts4"] = np.ascontiguousarray(pts4[:, c * n4c:(c + 1) * n4c])
        in_maps.append(m)
    return in_maps


def _numpy_fallback(points, cluster, num_clusters,
                    W0, b0, W1, b1, W2, b2, G0, g0, G1, g1):
    h = points.astype(np.float32)
    for W, b in ((W0, b0), (W1, b1), (W2, b2)):
        h = np.maximum(h @ W + b, 0.0)
    order = np.argsort(cluster, kind="stable")
    cs = cluster[order]
    hs = h[order]
    starts = np.searchsorted(cs, np.arange(num_clusters), side="left")
    counts = np.bincount(cs, minlength=num_clusters)
    safe_starts = np.minimum(starts, max(len(hs) - 1, 0))
    seg = np.maximum.reduceat(hs, safe_starts, axis=0)
    seg[counts == 0] = -np.inf   # match segment_max identity on empties
    pooled = seg
    gx = pooled
    for W, b in ((G0, g0), (G1, g1)):
        gx = np.maximum(gx @ W + b, 0.0)
    return gx.astype(np.float32)


def kernel(**inputs) -> np.ndarray:
    points = np.asarray(inputs["points"], np.float32)
    cluster = np.asarray(inputs["cluster"]).astype(np.int64)
    num_clusters = int(np.asarray(inputs["num_clusters"]))
    W0 = np.asarray(inputs["W0"], np.float32); b0 = np.asarray(inputs["b0"], np.float32)
    W1 = np.asarray(inputs["W1"], np.float32); b1 = np.asarray(inputs["b1"], np.float32)
    W2 = np.asarray(inputs["W2"], np.float32); b2 = np.asarray(inputs["b2"], np.float32)
    G0 = np.asarray(inputs["G0"], np.float32); g0 = np.asarray(inputs["g0"], np.float32)
    G1 = np.asarray(inputs["G1"], np.float32); g1 = np.asarray(inputs["g1"], np.float32)

    expected = (points.shape == (N, 3) and num_clusters == C
                and cluster.shape == (N,))
    if expected:
        # contiguous equal clusters of 32 points, as produced by setup_inputs
        expected = bool(
            np.array_equal(cluster[::PTS], np.arange(C, dtype=np.int64))
            and np.array_equal(cluster, np.repeat(cluster[::PTS], PTS))
        )
    if not expected:
        return _numpy_fallback(points, cluster, num_clusters,
                               W0, b0, W1, b1, W2, b2, G0, g0, G1, g1)

    from concourse.bass_utils import run_bass_kernel_spmd

    if "nc" not in _CACHE:
        _CACHE["nc"] = _build_module(N4C)
    nc = _CACHE["nc"]

    in_maps = _host_pack(points, W0, b0, W1, b1, W2, b2, G0, g0, G1, g1, N4C)
    res = run_bass_kernel_spmd(nc, in_maps, core_ids=list(range(NCORES)))
    outs = []
    for c in range(NCORES):
        o = np.asarray(res.results[c]["outt"]).astype(np.float32)  # [256, CPC]
        outs.append(o.T)                                           # [CPC, 256]
    return np.ascontiguousarray(np.concatenate(outs, axis=0))

